# revision 4
# baseline (speedup 1.0000x reference)
"""2D Haar DWT (DWT_2D) Trainium2 Bass kernel.

Input:  input [8, 64, 512, 512] f32 plus the four Haar DWT matrices.
Output: (LL, LH, HL, HH), each [8, 64, 256, 256] f32.

The Haar matrices have exactly two nonzeros (+-1/sqrt(2)) per row/col, so the
whole DWT is a 2x2 butterfly per input block:
    LL = 0.5*(a+b+c+d), LH = 0.5*(a-b+c-d),
    HL = 0.5*(a+b-c-d), HH = 0.5*(a-b-c+d)
with a=x[2i,2j], b=x[2i,2j+1], c=x[2i+1,2j], d=x[2i+1,2j+1]. The 0.5 scale is
folded into the host-side shard copy, and the reference's last-row/last-col
zero quirks (Hh row 255, mh1 col 255) are applied on the host after the
gather — the device does pure adds/subs plus DMA.

The kernel is DMA-bound: 16 DMA engines x ~25.8 GB/s each. All device I/O is
fp16 (the f32<->fp16 conversion rides on the host shard/gather copies), which
halves HBM traffic vs f32 at a ~5e-4 relative-error cost — far inside the
2e-2 gate.

Sharding: data-parallel over the batch dim, one batch element (64 slices of
[512,512]) per NeuronCore. Device kernel processes 4 slices per iteration:
one contiguous 2MB in-DMA (16KB per partition), vertical butterfly on DVE,
horizontal butterflies split DVE/GpSimd, two out-DMAs with 4KB-contiguous
runs per partition/band.
"""

import math
import os

import numpy as np

import concourse.bacc as bacc
import concourse.bass as bass
import concourse.mybir as mybir
from concourse.bass_utils import run_bass_kernel_spmd
from concourse.tile import TileContext

B, C, H, W = 8, 64, 512, 512
N_CORES = 8
SLICES_PER_CORE = (B * C) // N_CORES  # 64 [512,512] slices per core
PAIR = 4  # slices per device iteration
HD = mybir.dt.float16

_prog_cache = {}

# Set by test/profiling harnesses: when True, run_bass_kernel_spmd captures an
# NTFF profile and the BassKernelResults lands in LAST_RESULTS.
TRACE = False
LAST_RESULTS = None


def _build_program(n_slices: int) -> bass.Bass:
    # Bacc (not raw Bass): its compile() pass converts the Tile exit drain's
    # many sem waits into event semaphores; raw Bass fails walrus codegen
    # with "Too many sync wait commands".
    nc = bacc.Bacc(None, target_bir_lowering=False)
    x = nc.dram_tensor("x", [n_slices, H, W], HD, kind="ExternalInput")
    # All four subbands in one output tensor: [band, slice, 256, 256].
    out = nc.dram_tensor(
        "out", [4, n_slices, H // 2, W // 2], HD, kind="ExternalOutput"
    )

    n_iter = n_slices // PAIR
    rows_p = PAIR * H // 128  # input rows per partition (16)
    orow_p = rows_p // 2  # combined output rows per partition (8)
    parts_per_slice = H // rows_p  # 32
    # Input: PAIR slices = 2048 rows; partition p holds rows 16p..16p+15
    # (8 row pairs -> combined output rows 8p..8p+7). One contiguous 2MB DMA.
    x2 = x[:].rearrange("(i a) h w -> i (a h) w", a=PAIR)  # [i, 2048, 512]
    # Output: partition p = (a pp): slice a=p//32, rows 8*(p%32)+t, t<8.
    ov = out[:].rearrange(
        "b (i a) (pp t) w -> i (a pp) b t w", a=PAIR, t=orow_p
    )  # [i, 128, 4, 8, 256]

    with TileContext(nc) as tc:
        with tc.tile_pool(name="pool", bufs=1) as pool:
            for i in range(n_iter):
                # Alternate tile tags by iteration parity: the in-DMA for
                # iteration i+1 then writes a different SBUF region than the
                # tiles the compute engines are reading for iteration i,
                # dodging SBUF bank conflicts (measured 2.5x DVE slowdown when
                # DMA traffic lands adjacent to DVE operands).
                par = i % 2
                xt = pool.tile([128, rows_p, 512], HD, tag=f"xt{par}", bufs=2)
                # In-DMAs on the Sync sequencer; out-DMAs on the (otherwise
                # idle) Scalar sequencer so out-DMA waits can't
                # head-of-line-block in-DMA issue.
                nc.sync.dma_start(
                    out=xt[:], in_=x2[i].rearrange("(p q) w -> p q w", p=128)
                )

                xe = xt[:, 0:rows_p:2, :]  # even rows of the row pairs
                xo = xt[:, 1:rows_p:2, :]  # odd rows
                st = pool.tile([128, orow_p, 512], HD, tag=f"st{par}", bufs=1)
                dt = pool.tile([128, orow_p, 512], HD, tag=f"dt{par}", bufs=1)
                nc.vector.tensor_add(out=st[:], in0=xe, in1=xo)
                nc.vector.tensor_sub(out=dt[:], in0=xe, in1=xo)

                # Act engine (stable ~3.7us, otherwise idle) deinterleaves st
                # into contiguous even/odd column halves so the DVE horizontal
                # butterfly ops become half-size contiguous instead of
                # full-scan strided. dt stays interleaved: GpSimd's strided
                # adds read it directly (measured same cost either way).
                s01 = pool.tile(
                    [128, 2, orow_p, 256], HD, tag=f"s01{par}", bufs=1
                )
                nc.scalar.copy(
                    out=s01[:],
                    in_=st[:].rearrange("p r (j t) -> p t r j", t=2),
                )

                d0 = dt[:, :, 0:512:2]
                d1 = dt[:, :, 1:512:2]

                # Output tiles split by producer: DVE writes LL/LH, GpSimd
                # writes HL/HH. Each ships independently.
                oa = pool.tile([128, 2, orow_p, 256], HD, tag=f"oa{par}", bufs=1)
                ob = pool.tile([128, 2, orow_p, 256], HD, tag=f"ob{par}", bufs=1)
                nc.vector.tensor_add(out=oa[:, 0], in0=s01[:, 0], in1=s01[:, 1])
                nc.vector.tensor_sub(out=oa[:, 1], in0=s01[:, 0], in1=s01[:, 1])
                nc.gpsimd.tensor_add(out=ob[:, 0], in0=d0, in1=d1)  # HL
                nc.gpsimd.tensor_sub(out=ob[:, 1], in0=d0, in1=d1)  # HH

                nc.scalar.dma_start(out=ov[i][:, 0:2], in_=oa[:])
                nc.scalar.dma_start(out=ov[i][:, 2:4], in_=ob[:])
    nc.finalize()
    return nc


def _get_program(n_slices: int) -> bass.Bass:
    if n_slices not in _prog_cache:
        _prog_cache[n_slices] = _build_program(n_slices)
    return _prog_cache[n_slices]


def _expected_matrices():
    """Numpy port of reference.build_dwt_matrices for Haar, H=W=512."""
    sq = 1.0 / math.sqrt(2.0)
    ml0 = np.zeros((256, 512), np.float32)
    mh0 = np.zeros((256, 512), np.float32)
    for i in range(256):
        ml0[i, 2 * i : 2 * i + 2] = [sq, sq]
    for i in range(255):  # last row left zero (reference quirk)
        mh0[i, 2 * i : 2 * i + 2] = [sq, -sq]
    return ml0, ml0.T.copy(), mh0, mh0.T.copy()


def _numpy_fallback(x, ml0, ml1, mh0, mh1):
    out = []
    l = np.einsum("ih,bchw->bciw", ml0, x, optimize=True)
    hh_ = np.einsum("ih,bchw->bciw", mh0, x, optimize=True)
    for m in (l, hh_):
        for right in (ml1, mh1):
            out.append(np.einsum("bciw,wj->bcij", m, right, optimize=True))
    return tuple(np.ascontiguousarray(o.astype(np.float32)) for o in out)


def kernel(**inputs):
    x = np.asarray(inputs["input"], dtype=np.float32)
    assert x.shape == (B, C, H, W), x.shape

    ml0 = np.asarray(inputs["matrix_low_0"], dtype=np.float32)
    ml1 = np.asarray(inputs["matrix_low_1"], dtype=np.float32)
    mh0 = np.asarray(inputs["matrix_high_0"], dtype=np.float32)
    mh1 = np.asarray(inputs["matrix_high_1"], dtype=np.float32)
    el0, el1, eh0, eh1 = _expected_matrices()
    if not (
        np.array_equal(ml0, el0)
        and np.array_equal(ml1, el1)
        and np.array_equal(mh0, eh0)
        and np.array_equal(mh1, eh1)
    ):
        # Unexpected (non-Haar) matrices: stay correct via numpy.
        return _numpy_fallback(x, ml0, ml1, mh0, mh1)

    nc = _get_program(SLICES_PER_CORE)
    # The 0.5 DWT scale rides on the per-core shard copy, fused with the
    # fp16 downcast.
    xh = (0.5 * x.reshape(B * C, H, W)).astype(np.float16)
    in_maps = [
        {"x": xh[i * SLICES_PER_CORE : (i + 1) * SLICES_PER_CORE]}
        for i in range(N_CORES)
    ]
    global LAST_RESULTS
    try:
        res = run_bass_kernel_spmd(
            nc, in_maps, core_ids=list(range(N_CORES)), trace=TRACE
        )
    except ModuleNotFoundError:
        # A stray BASS_TRACE=1 in the environment routes through the NTFF
        # hook import, which this image lacks — retry untraced.
        os.environ["BASS_NEVER_TRACE"] = "1"
        res = run_bass_kernel_spmd(
            nc, in_maps, core_ids=list(range(N_CORES)), trace=False
        )
    LAST_RESULTS = res
    full = (
        np.concatenate([res.results[i]["out"] for i in range(N_CORES)], axis=1)
        .astype(np.float32)
        .reshape(4, B, C, H // 2, W // 2)
    )
    ll, lh, hl, hh = full[0], full[1], full[2], full[3]
    # Reference quirks: Hh row 255 == 0 (HL/HH row 255), mh1 col 255 == 0
    # (LH/HH col 255).
    lh[..., :, 255] = 0.0
    hl[..., 255, :] = 0.0
    hh[..., 255, :] = 0.0
    hh[..., :, 255] = 0.0
    return (ll, lh, hl, hh)


# revision 6
# speedup vs baseline: 1.0402x; 1.0402x over previous
"""2D Haar DWT (DWT_2D) Trainium2 Bass kernel.

Input:  input [8, 64, 512, 512] f32 plus the four Haar DWT matrices.
Output: (LL, LH, HL, HH), each [8, 64, 256, 256] f32.

The Haar matrices have exactly two nonzeros (+-1/sqrt(2)) per row/col, so the
whole DWT is a 2x2 butterfly per input block:
    LL = 0.5*(a+b+c+d), LH = 0.5*(a-b+c-d),
    HL = 0.5*(a+b-c-d), HH = 0.5*(a-b-c+d)
with a=x[2i,2j], b=x[2i,2j+1], c=x[2i+1,2j], d=x[2i+1,2j+1]. The 0.5 scale is
folded into the host-side shard copy, and the reference's last-row/last-col
zero quirks (Hh row 255, mh1 col 255) are applied on the host after the
gather — the device does pure adds/subs plus DMA.

The kernel is DMA-bound: 16 DMA engines x ~25.8 GB/s each. All device I/O is
fp16 (the f32<->fp16 conversion rides on the host shard/gather copies), which
halves HBM traffic vs f32 at a ~5e-4 relative-error cost — far inside the
2e-2 gate.

Sharding: data-parallel over the batch dim, one batch element (64 slices of
[512,512]) per NeuronCore. Device kernel processes 4 slices per iteration:
one contiguous 2MB in-DMA (16KB per partition), vertical butterfly on DVE,
horizontal butterflies split DVE/GpSimd, two out-DMAs with 4KB-contiguous
runs per partition/band.
"""

import math
import os

import numpy as np

import concourse.bacc as bacc
import concourse.bass as bass
import concourse.mybir as mybir
from concourse.bass_utils import run_bass_kernel_spmd
from concourse.tile import TileContext

B, C, H, W = 8, 64, 512, 512
N_CORES = 8
SLICES_PER_CORE = (B * C) // N_CORES  # 64 [512,512] slices per core
PAIR = 8  # slices per device iteration
HD = mybir.dt.float16

_prog_cache = {}

# Set by test/profiling harnesses: when True, run_bass_kernel_spmd captures an
# NTFF profile and the BassKernelResults lands in LAST_RESULTS.
TRACE = False
LAST_RESULTS = None


def _build_program(n_slices: int) -> bass.Bass:
    # Bacc (not raw Bass): its compile() pass converts the Tile exit drain's
    # many sem waits into event semaphores; raw Bass fails walrus codegen
    # with "Too many sync wait commands".
    nc = bacc.Bacc(None, target_bir_lowering=False)
    x = nc.dram_tensor("x", [n_slices, H, W], HD, kind="ExternalInput")
    # All four subbands in one output tensor: [band, slice, 256, 256].
    out = nc.dram_tensor(
        "out", [4, n_slices, H // 2, W // 2], HD, kind="ExternalOutput"
    )

    n_iter = n_slices // PAIR
    rows_p = PAIR * H // 128  # input rows per partition (16)
    orow_p = rows_p // 2  # combined output rows per partition (8)
    parts_per_slice = H // rows_p  # 32
    # Input: PAIR slices = 2048 rows; partition p holds rows 16p..16p+15
    # (8 row pairs -> combined output rows 8p..8p+7). One contiguous 2MB DMA.
    x2 = x[:].rearrange("(i a) h w -> i (a h) w", a=PAIR)  # [i, 2048, 512]
    # Output: partition p = (a pp): slice a=p//32, rows 8*(p%32)+t, t<8.
    ov = out[:].rearrange(
        "b (i a) (pp t) w -> i (a pp) b t w", a=PAIR, t=orow_p
    )  # [i, 128, 4, 8, 256]

    with TileContext(nc) as tc:
        with tc.tile_pool(name="pool", bufs=1) as pool:
            for i in range(n_iter):
                # Big ops amortize the fixed ~4.3us SBUF-contention penalty a
                # compute op pays when it overlaps active DMA traffic.
                xt = pool.tile([128, rows_p, 512], HD, tag="xt", bufs=2)
                # In-DMAs on the Sync sequencer; out-DMAs on the (otherwise
                # idle) Scalar sequencer so out-DMA waits can't
                # head-of-line-block in-DMA issue.
                nc.sync.dma_start(
                    out=xt[:], in_=x2[i].rearrange("(p q) w -> p q w", p=128)
                )

                xe = xt[:, 0:rows_p:2, :]  # even rows of the row pairs
                xo = xt[:, 1:rows_p:2, :]  # odd rows
                st = pool.tile([128, orow_p, 512], HD, tag="st", bufs=2)
                dt = pool.tile([128, orow_p, 512], HD, tag="dt", bufs=2)
                nc.vector.tensor_add(out=st[:], in0=xe, in1=xo)
                nc.vector.tensor_sub(out=dt[:], in0=xe, in1=xo)

                s0 = st[:, :, 0:512:2]
                s1 = st[:, :, 1:512:2]
                d0 = dt[:, :, 0:512:2]
                d1 = dt[:, :, 1:512:2]

                # Output tiles split by producer: DVE writes LL/LH, GpSimd
                # writes HL/HH. Each ships independently.
                oa = pool.tile([128, 2, orow_p, 256], HD, tag="oa", bufs=2)
                ob = pool.tile([128, 2, orow_p, 256], HD, tag="ob", bufs=2)
                nc.vector.tensor_add(out=oa[:, 0], in0=s0, in1=s1)  # LL
                nc.vector.tensor_sub(out=oa[:, 1], in0=s0, in1=s1)  # LH
                nc.gpsimd.tensor_add(out=ob[:, 0], in0=d0, in1=d1)  # HL
                nc.gpsimd.tensor_sub(out=ob[:, 1], in0=d0, in1=d1)  # HH

                nc.scalar.dma_start(out=ov[i][:, 0:2], in_=oa[:])
                nc.scalar.dma_start(out=ov[i][:, 2:4], in_=ob[:])
    nc.finalize()
    return nc


def _get_program(n_slices: int) -> bass.Bass:
    if n_slices not in _prog_cache:
        _prog_cache[n_slices] = _build_program(n_slices)
    return _prog_cache[n_slices]


def _expected_matrices():
    """Numpy port of reference.build_dwt_matrices for Haar, H=W=512."""
    sq = 1.0 / math.sqrt(2.0)
    ml0 = np.zeros((256, 512), np.float32)
    mh0 = np.zeros((256, 512), np.float32)
    for i in range(256):
        ml0[i, 2 * i : 2 * i + 2] = [sq, sq]
    for i in range(255):  # last row left zero (reference quirk)
        mh0[i, 2 * i : 2 * i + 2] = [sq, -sq]
    return ml0, ml0.T.copy(), mh0, mh0.T.copy()


def _numpy_fallback(x, ml0, ml1, mh0, mh1):
    out = []
    l = np.einsum("ih,bchw->bciw", ml0, x, optimize=True)
    hh_ = np.einsum("ih,bchw->bciw", mh0, x, optimize=True)
    for m in (l, hh_):
        for right in (ml1, mh1):
            out.append(np.einsum("bciw,wj->bcij", m, right, optimize=True))
    return tuple(np.ascontiguousarray(o.astype(np.float32)) for o in out)


def kernel(**inputs):
    x = np.asarray(inputs["input"], dtype=np.float32)
    assert x.shape == (B, C, H, W), x.shape

    ml0 = np.asarray(inputs["matrix_low_0"], dtype=np.float32)
    ml1 = np.asarray(inputs["matrix_low_1"], dtype=np.float32)
    mh0 = np.asarray(inputs["matrix_high_0"], dtype=np.float32)
    mh1 = np.asarray(inputs["matrix_high_1"], dtype=np.float32)
    el0, el1, eh0, eh1 = _expected_matrices()
    if not (
        np.array_equal(ml0, el0)
        and np.array_equal(ml1, el1)
        and np.array_equal(mh0, eh0)
        and np.array_equal(mh1, eh1)
    ):
        # Unexpected (non-Haar) matrices: stay correct via numpy.
        return _numpy_fallback(x, ml0, ml1, mh0, mh1)

    nc = _get_program(SLICES_PER_CORE)
    # The 0.5 DWT scale rides on the per-core shard copy, fused with the
    # fp16 downcast.
    xh = (0.5 * x.reshape(B * C, H, W)).astype(np.float16)
    in_maps = [
        {"x": xh[i * SLICES_PER_CORE : (i + 1) * SLICES_PER_CORE]}
        for i in range(N_CORES)
    ]
    global LAST_RESULTS
    try:
        res = run_bass_kernel_spmd(
            nc, in_maps, core_ids=list(range(N_CORES)), trace=TRACE
        )
    except ModuleNotFoundError:
        # A stray BASS_TRACE=1 in the environment routes through the NTFF
        # hook import, which this image lacks — retry untraced.
        os.environ["BASS_NEVER_TRACE"] = "1"
        res = run_bass_kernel_spmd(
            nc, in_maps, core_ids=list(range(N_CORES)), trace=False
        )
    LAST_RESULTS = res
    full = (
        np.concatenate([res.results[i]["out"] for i in range(N_CORES)], axis=1)
        .astype(np.float32)
        .reshape(4, B, C, H // 2, W // 2)
    )
    ll, lh, hl, hh = full[0], full[1], full[2], full[3]
    # Reference quirks: Hh row 255 == 0 (HL/HH row 255), mh1 col 255 == 0
    # (LH/HH col 255).
    lh[..., :, 255] = 0.0
    hl[..., 255, :] = 0.0
    hh[..., 255, :] = 0.0
    hh[..., :, 255] = 0.0
    return (ll, lh, hl, hh)
